# revision 11
# baseline (speedup 1.0000x reference)
"""CBOW embedding-lookup kernel for Trainium2 (8 NeuronCores).

Math: out[b, o] = sum_i fc_w[o, i*V + contexts[b, i]] + fc_b[o]
i.e. a row-gather over a transposed view of the fc weight, summed over the
C=4 context slots, plus bias.

Strategy (BATCH_WAYS x VOCAB_WAYS = 8 cores; 4 x 2 by default):
  - Host: build table t[i, v, o] = fc_w[o, i*V+v] + fc_b[o]/C, shard o into
    VOCAB_WAYS contiguous column blocks -> per-core contiguous table
    [C*V, V/VOCAB_WAYS] f32. Folding bias/C into every row makes the device
    work a pure gather + 3 adds per output row.
  - Device: each core owns B/BATCH_WAYS batch rows and V/VOCAB_WAYS output
    cols. Per 128-row batch block: indirect-DMA gathers (one line-rate
    descriptor per row; CCE-accumulate was measured 2x slower per
    descriptor, so the reduction runs on VectorE instead), a chained DVE
    reduction, and a DMA store. Pair-first issue order: slots 0+1 of every
    block stream in first so the DVE chain starts early; later slots' adds
    chase their gathers while other blocks keep the 16 SDMA engines
    saturated. Only the last block's final add + store sit in the tail.
  - Host: stitch the 8 per-core outputs into [B, V].
"""

import os

import numpy as np

from concourse import bacc, bass, mybir
import concourse.tile as tile
from concourse.bass_utils import run_bass_kernel_spmd

V = 8192          # vocab (both in and out)
C = 4             # context slots
B = 1024          # batch
M = 8             # cores
P = 128           # SBUF partitions / batch block
R = C * V         # table rows

BATCH_WAYS = int(os.environ.get("KERNEL_BATCH_WAYS", "4"))
VOCAB_WAYS = M // BATCH_WAYS
BS = B // BATCH_WAYS   # batch rows per core
VS = V // VOCAB_WAYS   # output cols per core
NBLK = BS // P         # 128-row batch blocks per core

_NC_CACHE = None
LAST_RESULTS = None  # test harness reads exec_time_ns from here


def _build_nc():
    nc = bacc.Bacc("TRN2", target_bir_lowering=False, debug=False)
    idx_d = nc.dram_tensor("idx", [BS, C], mybir.dt.int32, kind="ExternalInput")
    tab_d = nc.dram_tensor("tab", [R, VS], mybir.dt.float32, kind="ExternalInput")
    out_d = nc.dram_tensor("out", [BS, VS], mybir.dt.float32, kind="ExternalOutput")

    with tile.TileContext(nc) as tc:
        with tc.tile_pool(name="sbuf", bufs=1) as pool:
            idx_ts, slots, accs = [], [], []
            for blk in range(NBLK):
                row0 = blk * P
                idx_t = pool.tile([P, C], mybir.dt.int32, tag=f"idx{blk}")
                nc.sync.dma_start(out=idx_t[:], in_=idx_d[row0 : row0 + P, :])
                idx_ts.append(idx_t)
                # one tile per (block, slot): no shared-tile WAR deps between
                # late gathers and the DVE reads of earlier slots
                slots.append(
                    [
                        pool.tile(
                            [P, VS],
                            mybir.dt.float32,
                            tag=f"g{blk}_{i}",
                            name=f"g{blk}_{i}",
                        )
                        for i in range(C)
                    ]
                )
                accs.append(
                    pool.tile([P, VS], mybir.dt.float32, tag=f"a{blk}", name=f"a{blk}")
                )

            def gather(blk, i):
                # NB: a multi-column offset AP ([P, C] indices in one op)
                # passes CoreSim but returns garbage on HW — keep [P, 1].
                nc.gpsimd.indirect_dma_start(
                    out=slots[blk][i][:],
                    out_offset=None,
                    in_=tab_d[:],
                    in_offset=bass.IndirectOffsetOnAxis(
                        ap=idx_ts[blk][:, i : i + 1], axis=0
                    ),
                )

            # Pair-first issue: slots 0+1 of each block stream in first so the
            # DVE reduction starts as early as possible.
            for blk in range(NBLK):
                gather(blk, 0)
                gather(blk, 1)
            for blk in range(NBLK):
                nc.vector.tensor_add(
                    out=accs[blk][:], in0=slots[blk][0][:], in1=slots[blk][1][:]
                )
            tail_split = bool(int(os.environ.get("KERNEL_TAIL_SPLIT", "0")))
            last = NBLK - 1
            for i in range(2, C):
                for blk in range(NBLK):
                    gather(blk, i)
                for blk in range(NBLK):
                    if tail_split and i == C - 1 and blk == last:
                        continue  # handled below in halves
                    nc.vector.tensor_add(
                        out=accs[blk][:], in0=accs[blk][:], in1=slots[blk][i][:]
                    )
            for blk in range(NBLK):
                row0 = blk * P
                if tail_split and blk == last:
                    continue
                nc.sync.dma_start(out=out_d[row0 : row0 + P, :], in_=accs[blk][:])
            if tail_split:
                # the last block's final add + store leave the critical path in
                # half-width pieces: store of half 0 overlaps the add of half 1
                row0 = last * P
                vh = VS // 2
                for half in range(2):
                    sl = slice(half * vh, (half + 1) * vh)
                    nc.vector.tensor_add(
                        out=accs[last][:, sl],
                        in0=accs[last][:, sl],
                        in1=slots[last][C - 1][:, sl],
                    )
                    nc.sync.dma_start(
                        out=out_d[row0 : row0 + P, sl], in_=accs[last][:, sl]
                    )
    nc.compile()
    return nc


def _host_prep(contexts, fc_w, fc_b):
    contexts = np.asarray(contexts)
    fc_w = np.asarray(fc_w, dtype=np.float32)
    fc_b = np.asarray(fc_b, dtype=np.float32)
    idx = np.arange(C, dtype=np.int32)[None, :] * V + contexts.astype(np.int32)
    idx = np.ascontiguousarray(idx)

    w3 = fc_w.reshape(V, C, V)  # [o, i, v]
    bias_per_slot = (fc_b / C)[:, None]  # [o, 1]
    vocab_shards = []
    for vw in range(VOCAB_WAYS):
        o_sl = slice(vw * VS, (vw + 1) * VS)
        shard = np.empty((C, V, VS), dtype=np.float32)
        for i in range(C):
            # [o_shard, v].T -> [v, o_shard], fused bias add
            np.add(w3[o_sl, i, :].T, bias_per_slot[o_sl].T, out=shard[i])
        vocab_shards.append(shard.reshape(R, VS))
    return idx, vocab_shards


def kernel(contexts, fc_w, fc_b):
    global _NC_CACHE, LAST_RESULTS
    idx, vocab_shards = _host_prep(contexts, fc_w, fc_b)
    if _NC_CACHE is None:
        _NC_CACHE = _build_nc()
    nc = _NC_CACHE

    # core m = bw * VOCAB_WAYS + vw owns batch rows [bw*BS:(bw+1)*BS] and
    # output cols [vw*VS:(vw+1)*VS]
    in_maps = []
    for m in range(M):
        bw, vw = divmod(m, VOCAB_WAYS)
        in_maps.append(
            {"idx": idx[bw * BS : (bw + 1) * BS], "tab": vocab_shards[vw]}
        )
    trace = bool(os.environ.get("KERNEL_TRACE"))
    res = run_bass_kernel_spmd(
        nc, in_maps, list(range(M)), trace=trace, stitch_traces=False
    )
    LAST_RESULTS = res

    out = np.empty((B, V), dtype=np.float32)
    for m in range(M):
        bw, vw = divmod(m, VOCAB_WAYS)
        out[bw * BS : (bw + 1) * BS, vw * VS : (vw + 1) * VS] = res.results[m]["out"]
    return out
